# revision 17
# baseline (speedup 1.0000x reference)
"""Trainium2 Bass kernel for nn_AttentionLayer (B=8, S=2048, D=256, U=128).

Data-parallel over batch: one batch element per NeuronCore, weights replicated.
Per-core flash-attention layer, v4.

Sequence relabeling: row s of X lives at (partition p, tile t) with
s = p*NT + t (contiguous multi-KB DMA runs per partition).  Attention is
permutation-invariant over sequence position as long as loads, K/V indexing,
residual, and stores use the same relabeling (they do).

v4 design notes:
  - Queries processed in 4 chunks of 512; k-loop per chunk runs 16 k-tiles
    as 8 pair-iterations with a [128,1024] exp per pair.  Only the last
    chunk's finish is exposed at the end.
  - x loaded as f32 over the (otherwise idle) SP HWDGE queue; transposes
    are f32 transpose-mode matmuls; the PSUM->SBUF copy casts to fp8.
  - Q/K/V projections are single-instruction fp8 DoubleRow matmuls
    (256-deep contraction).  W_q/W_k/W_v pre-scaled by 8 into fp8 e4m3
    range; the 64x score scale folds into the exp scale, the 8x V scale
    folds into the reciprocal (rowsum-transpose ones value 8.0).
    NOTE: multi-instruction DoubleRow PSUM accumulation is broken on this
    hardware when any non-DoubleRow matmul lands between group members
    (the scheduler interleaves freely), so DR is used ONLY with
    start=stop=True.  AV accumulation stays plain f16.
  - exp on ScalarE -> fp16 e tiles [128,2,512]: the pace-setter.
  - Row-sums split: columns [0,384) accumulate on DVE (one [128,2,384]
    fp16 2x-mode add per pair), columns [384,512) via a [128,1] f16
    ones-matmul per k-tile into a [1,128] PSUM accumulator on the PE.
  - Per-chunk finish: otb copy, rowsum transposes (K=1 matmuls), recip,
    4x proj matmul + scalar_tensor_tensor (*recip + residual), stores.
    Interleaved into the next chunk's k-loop.
  - Residual xres = x + b_o precomputed on GpSimd in f32 (idle engine).
"""

import os
import sys

if "/opt/trn_rl_repo" not in sys.path:
    sys.path.insert(0, "/opt/trn_rl_repo")

from contextlib import ExitStack

import numpy as np

import concourse.bass as bass
import concourse.tile as tile
from concourse import bacc, mybir
from concourse.bass_utils import run_bass_kernel_spmd
from concourse.masks import make_identity

B, S, D, U, P = 8, 2048, 256, 128, 128
NT = S // P            # 16 sequence tiles of 128
CH = 512               # query chunk width
NCH = S // CH          # 4 chunks
NPAIR = NT // 2        # 8 kt-pair iterations per chunk
RD = 384               # racc covers columns [0, RD); PE rowsum [RD, CH)
W_SCALE = 8.0          # fp8 pre-scale on W_q/W_k/W_v
EXP_SCALE = 1.0 / (float(np.sqrt(U)) * W_SCALE * W_SCALE)
F32 = mybir.dt.float32
F16 = mybir.dt.float16
F8 = mybir.dt.float8e4
DR = mybir.MatmulPerfMode.DoubleRow
EXP = mybir.ActivationFunctionType.Exp
E_BUFS = 6             # e-tile ring
LAG = 2                # AV/racc lag in pair-iterations
N_WARMUP = 38

DEBUG = bool(os.environ.get("BASSDBG"))


def build_bass():
    nc = bacc.Bacc("TRN2", target_bir_lowering=False, debug=False)

    x = nc.dram_tensor("inputs", [S, D], F32, kind="ExternalInput").ap()
    wq_d = nc.dram_tensor("W_q", [D, U], F32, kind="ExternalInput").ap()
    wk_d = nc.dram_tensor("W_k", [D, U], F32, kind="ExternalInput").ap()
    wv_d = nc.dram_tensor("W_v", [D, U], F32, kind="ExternalInput").ap()
    wo_d = nc.dram_tensor("W_o", [U, D], F32, kind="ExternalInput").ap()
    bo_d = nc.dram_tensor("b_o", [D], F32, kind="ExternalInput").ap()
    out_d = nc.dram_tensor("out", [S, D], F16, kind="ExternalOutput").ap()
    if DEBUG:
        dbg_qt = nc.dram_tensor("dbg_qt", [P, S], F16, kind="ExternalOutput").ap()
        dbg_kt = nc.dram_tensor("dbg_kt", [P, S], F16, kind="ExternalOutput").ap()
        dbg_v = nc.dram_tensor("dbg_v", [P, NT * U], F16, kind="ExternalOutput").ap()
        dbg_rt = nc.dram_tensor("dbg_rt", [P, 4], F32, kind="ExternalOutput").ap()
        dbg_ot = nc.dram_tensor("dbg_ot", [P, CH], F32, kind="ExternalOutput").ap()

    x_tiled = x.rearrange("(p t) d -> p t d", t=NT)
    out_tiled = out_d.rearrange("(p t) d -> p t d", t=NT)

    with tile.TileContext(nc) as tc, ExitStack() as ctx:
        consts = ctx.enter_context(tc.tile_pool(name="consts", bufs=1))
        sb = ctx.enter_context(tc.tile_pool(name="sb", bufs=1))
        work = ctx.enter_context(tc.tile_pool(name="work", bufs=E_BUFS))
        outp = ctx.enter_context(tc.tile_pool(name="outp", bufs=2))
        # PSUM (8 banks): sc 2x[128,2,512]f32 = 4, ot 1x[128,512] = 1,
        # rs 1x[1,128] = 1, misc 2x(1 bank) = 2.
        ps_sc = ctx.enter_context(tc.tile_pool(name="ps_sc", bufs=2, space="PSUM"))
        ps_ot = ctx.enter_context(tc.tile_pool(name="ps_ot", bufs=1, space="PSUM"))
        ps_rs = ctx.enter_context(tc.tile_pool(name="ps_rs", bufs=1, space="PSUM"))
        ps_misc = ctx.enter_context(tc.tile_pool(name="ps_misc", bufs=2, space="PSUM"))

        # ---- tiny constants ----
        wu_sb = consts.tile([P, P], F16)
        nc.vector.memset(wu_sb, 0.0)
        zbias = consts.tile([P, 1], F32)
        nc.vector.memset(zbias, 0.0)
        ones16 = consts.tile([P, 1], F16)
        nc.vector.memset(ones16, 1.0)
        one1 = consts.tile([1, 1], F16)   # rs16 transpose rhs: folds the 8x
        nc.vector.memset(one1, W_SCALE)
        ones8v = consts.tile([P, 1], F16)  # racc transpose rhs: folds the 8x
        nc.vector.memset(ones8v, W_SCALE)

        ident32 = consts.tile([P, P], F32)
        make_identity(nc, ident32)

        # ---- loads: weights first (small), then x chunks; all f32 HWDGE ----
        wq_f = consts.tile([P, 2, U], F32)
        wk_f = consts.tile([P, 2, U], F32)
        wv_f = consts.tile([P, 2, U], F32)
        wo_f = consts.tile([P, D], F32)
        x32 = sb.tile([P, NT, D], F32)
        nc.sync.dma_start(out=wq_f[:], in_=wq_d.rearrange("(c p) u -> p c u", p=P))
        nc.sync.dma_start(out=wk_f[:], in_=wk_d.rearrange("(c p) u -> p c u", p=P))
        nc.sync.dma_start(out=x32[:, 0:2, :], in_=x_tiled[:, 0:2, :])
        nc.sync.dma_start(out=x32[:, 2:4, :], in_=x_tiled[:, 2:4, :])
        nc.sync.dma_start(out=wv_f[:], in_=wv_d.rearrange("(c p) u -> p c u", p=P))
        nc.sync.dma_start(out=x32[:, 4:8, :], in_=x_tiled[:, 4:8, :])
        nc.sync.dma_start(out=wo_f[:], in_=wo_d)
        nc.sync.dma_start(out=x32[:, 8:12, :], in_=x_tiled[:, 8:12, :])
        nc.sync.dma_start(out=x32[:, 12:16, :], in_=x_tiled[:, 12:16, :])

        # b_o broadcast (f32, no cast) on gpsimd
        bo32 = consts.tile([P, D], F32)
        bo_bcast = bass.AP(tensor=bo_d.tensor, offset=bo_d.offset,
                           ap=[[0, P]] + list(bo_d.ap))
        nc.gpsimd.dma_start(out=bo32[:], in_=bo_bcast)

        # W casts: wq8/wk8/wv8 = 8*W in fp8 (Scalar), wo16 f16 (DVE)
        wq8 = consts.tile([P, 2, U], F8)
        wk8 = consts.tile([P, 2, U], F8)
        wv8 = consts.tile([P, 2, U], F8)
        wo16 = consts.tile([P, D], F16)
        nc.scalar.mul(wq8, wq_f, W_SCALE)
        nc.scalar.mul(wk8, wk_f, W_SCALE)
        nc.scalar.mul(wv8, wv_f, W_SCALE)
        nc.vector.tensor_copy(wo16[:], wo_f[:])

        # ---- PE warmup (HAM ramp) ----
        wu_ps = ps_misc.tile([P, P], F32, tag="m")
        for _ in range(N_WARMUP):
            nc.tensor.matmul(wu_ps[:], wu_sb[:], wu_sb[:], start=True, stop=True)

        def pe_fill(n):
            for _ in range(n):
                nc.tensor.matmul(wu_ps[:], wu_sb[:], wu_sb[:], start=True,
                                 stop=True)

        # ---- residual xres = x + b_o on GpSimd (f32) ----
        xres32 = sb.tile([P, NT, D], F32)
        bo_rep = bass.AP(tensor=bo32.tensor, offset=bo32.offset,
                         ap=[list(bo32.ap[0]), [0, 2]] + list(bo32.ap[1:]))

        def emit_xres(g):
            for i in range(2):
                t0 = 4 * g + 2 * i
                nc.gpsimd.tensor_add(xres32[:, t0:t0 + 2, :],
                                     x32[:, t0:t0 + 2, :], bo_rep)

        # ---- QKV building blocks ----
        xt8 = sb.tile([P, 2, NT, P], F8)   # X^T: [d_in_chunk, c, t, s]
        xt2 = xt8.rearrange("p c t s -> p c (t s)")
        qt = sb.tile([P, S], F16)          # 8*Q^T [u, q-col]
        kt16 = sb.tile([P, S], F16)        # 8*K^T [u, k-col]
        v16 = sb.tile([P, NT, U], F16)     # 8*V natural [s_in_tile, t, u]

        def tx_half(g, c, copy_eng):
            # 4 f32 transpose-mode matmuls into one misc bank; copy casts
            # f32 -> fp8
            txg = ps_misc.tile([P, 4, P], F32, tag="m")
            for dt in range(4):
                t = 4 * g + dt
                nc.tensor.matmul(txg[:, dt, :],
                                 x32[:, t, c * P:(c + 1) * P],
                                 ident32[:], is_transpose=True,
                                 start=True, stop=True)
            copy_eng(xt8[:, c, 4 * g:4 * g + 4, :], txg[:])

        def qk_group(g, w8, dst, copy_eng):
            # single-instruction DoubleRow projections (256-deep over d)
            ps = ps_misc.tile([P, CH], F32, tag="m")
            for h in range(2):
                csl = slice(g * CH + h * 256, g * CH + (h + 1) * 256)
                nc.tensor.matmul(ps[:, h * 256:(h + 1) * 256],
                                 w8[:], xt2[:, :, csl],
                                 perf_mode=DR, start=True, stop=True)
            copy_eng(dst[:, g * CH:(g + 1) * CH], ps[:])

        def v_group(g, copy_eng):
            vg = ps_misc.tile([P, 4, U], F32, tag="m")
            for dt in range(4):
                t = 4 * g + dt
                nc.tensor.matmul(vg[:, dt, :], xt8[:, :, t, :], wv8[:],
                                 perf_mode=DR, start=True, stop=True)
            copy_eng(v16[:, 4 * g:4 * g + 4, :], vg[:])

        # ---- per-chunk attention ----
        class ChunkState:
            pass

        def begin_chunk(c):
            st = ChunkState()
            st.c = c
            st.q = slice(c * CH, (c + 1) * CH)
            st.ot = ps_ot.tile([P, CH], F32, tag="ot")
            st.rs = ps_rs.tile([1, P], F32, tag="rs")
            st.racc = outp.tile([P, 2, RD], F16, tag="racc")
            st.pend = []
            return st

        def pair_scores(st, pr):
            sc = ps_sc.tile([P, 2, CH], F32, tag="sc")
            for j in range(2):
                kt = 2 * pr + j
                ksl = slice(kt * P, (kt + 1) * P)
                nc.tensor.matmul(sc[:, j, :], kt16[:, ksl], qt[:, st.q],
                                 start=True, stop=True)
            e = work.tile([P, 2, CH], F16, tag="e")
            nc.scalar.activation(e[:], sc[:], EXP, bias=zbias[:],
                                 scale=EXP_SCALE)
            return e

        def pair_av(st, pr, e):
            for j in range(2):
                kt = 2 * pr + j
                first = kt == 0
                last = kt == NT - 1
                nc.tensor.matmul(st.ot[:], v16[:, kt, :], e[:, j, :],
                                 start=first, stop=last)
                nc.tensor.matmul(st.rs[:], ones16[:], e[:, j, RD:CH],
                                 start=first, stop=last)
            if pr == 0:
                nc.vector.tensor_copy(st.racc[:], e[:, :, 0:RD])
            else:
                nc.vector.tensor_add(st.racc[:], st.racc[:], e[:, :, 0:RD])

        def kloop(st, prs, extra=None, flush=False):
            prs = list(prs)
            for i, pr in enumerate(prs):
                e = pair_scores(st, pr)
                st.pend.append((pr, e))
                while len(st.pend) > LAG:
                    p2, e2 = st.pend.pop(0)
                    pair_av(st, p2, e2)
                if extra:
                    want = -(-len(extra) // (len(prs) - i))
                    for _ in range(want):
                        extra.pop(0)()
            if flush:
                for p2, e2 in st.pend:
                    pair_av(st, p2, e2)
                st.pend = []

        def finish_chunk(st, tail):
            c = st.c
            otb = outp.tile([P, CH], F16, tag="otb")
            rs16 = outp.tile([1, P], F16, tag="rs16")
            rtT = ps_misc.tile([P, 4], F32, tag="m")
            recip = outp.tile([P, 4], F32, tag="recip")
            obuf = outp.tile([P, 4, D], F16, tag="obuf")
            thunks = []
            if tail:
                thunks.append(lambda: nc.scalar.copy(otb[:], st.ot[:]))
                thunks.append(lambda: nc.scalar.copy(rs16[:], st.rs[:]))
            else:
                thunks.append(lambda: nc.vector.tensor_copy(otb[:], st.ot[:]))
                thunks.append(lambda: nc.vector.tensor_copy(rs16[:], st.rs[:]))

            def rowsum_transpose():
                # cols [0,RD): partition-sum of racc via K=128 ones matmuls
                # (both kt sub-rows accumulate); cols [RD,CH): transpose of
                # the PE rowsum row.  one1 = 8.0 folds the V scale.
                for j in range(3):
                    jb = slice(j * P, (j + 1) * P)
                    nc.tensor.matmul(rtT[:, j:j + 1], st.racc[:, 0, jb],
                                     ones8v[:], start=True, stop=False)
                    nc.tensor.matmul(rtT[:, j:j + 1], st.racc[:, 1, jb],
                                     ones8v[:], start=False, stop=True)
                nc.tensor.matmul(rtT[:, 3:4], rs16[:], one1[:],
                                 start=True, stop=True)
                nc.vector.reciprocal(recip[:], rtT[:])

            thunks.append(rowsum_transpose)

            def proj(j):
                t = 4 * c + j
                pj = ps_misc.tile([P, D], F32, tag="m")
                nc.tensor.matmul(pj[:], otb[:, j * P:(j + 1) * P], wo16[:],
                                 start=True, stop=True)
                nc.vector.scalar_tensor_tensor(
                    obuf[:, j, :], pj[:], recip[:, j:j + 1],
                    xres32[:, t, :], op0=mybir.AluOpType.mult,
                    op1=mybir.AluOpType.add)
                if j % 2 == 1:
                    nc.sync.dma_start(
                        out=out_tiled[:, t - 1:t + 1, :],
                        in_=obuf[:, j - 1:j + 1, :])

            for j in range(4):
                thunks.append(lambda j=j: proj(j))
            return thunks

        # ---- schedule ----
        # Pre-stream: transposes g0-g2 + Q/K/V of g0 (Scalar does the big
        # copies; it is free until the exp stream starts).
        pe_fill(4)
        tx_half(0, 0, nc.scalar.copy)
        tx_half(0, 1, nc.scalar.copy)
        pe_fill(4)
        qk_group(0, wq8, qt, nc.scalar.copy)
        qk_group(0, wk8, kt16, nc.vector.tensor_copy)
        v_group(0, nc.vector.tensor_copy)
        tx_half(1, 0, nc.scalar.copy)
        tx_half(1, 1, nc.scalar.copy)
        qk_group(1, wk8, kt16, nc.vector.tensor_copy)

        st0 = begin_chunk(0)
        g_extras = [
            lambda: v_group(1, nc.vector.tensor_copy),
            lambda: tx_half(2, 0, nc.vector.tensor_copy),
            lambda: tx_half(2, 1, nc.vector.tensor_copy),
            lambda: qk_group(2, wk8, kt16, nc.vector.tensor_copy),
            lambda: tx_half(3, 0, nc.vector.tensor_copy),
            lambda: tx_half(3, 1, nc.vector.tensor_copy),
            lambda: qk_group(3, wk8, kt16, nc.vector.tensor_copy),
            lambda: v_group(2, nc.vector.tensor_copy),
            lambda: v_group(3, nc.vector.tensor_copy),
            lambda: qk_group(1, wq8, qt, nc.vector.tensor_copy),
        ]
        kloop(st0, range(NPAIR), extra=g_extras, flush=True)
        if DEBUG:
            nc.sync.dma_start(out=dbg_qt, in_=qt[:])
            nc.sync.dma_start(out=dbg_kt, in_=kt16[:])
            v16d = sb.tile([P, NT * U], F16, name="v16d")
            nc.vector.tensor_copy(v16d[:], v16.rearrange("p t u -> p (t u)"))
            nc.sync.dma_start(out=dbg_v, in_=v16d[:])
            otd = sb.tile([P, CH], F32, name="otd")
            nc.vector.tensor_copy(otd[:], st0.ot[:])
            nc.sync.dma_start(out=dbg_ot, in_=otd[:])
        fin = finish_chunk(st0, tail=False)
        if DEBUG:
            # rowsum transposes live in fin; dump rtT after chunk0's fin by
            # re-deriving from racc is awkward -- instead dump recip later via
            # obuf comparisons.  Here dump rtT once chunk1 runs fin thunks.
            pass

        emit_xres(0)
        emit_xres(1)

        for c in range(1, NCH):
            st = begin_chunk(c)
            extras = list(fin)
            if c == 1:
                extras.append(lambda: qk_group(2, wq8, qt,
                                               nc.vector.tensor_copy))
                emit_xres(2)
                emit_xres(3)
            if c == 2:
                extras.append(lambda: qk_group(3, wq8, qt,
                                               nc.vector.tensor_copy))
            kloop(st, range(NPAIR), extra=extras, flush=True)
            fin = finish_chunk(st, tail=(c == NCH - 1))
        for th in fin:
            th()

    nc.compile()
    return nc


_NC_CACHE = None


def _get_nc():
    global _NC_CACHE
    if _NC_CACHE is None:
        _NC_CACHE = build_bass()
    return _NC_CACHE


def make_in_maps(inputs, W_q, W_k, W_v, W_o, b_o):
    return [
        {
            "inputs": np.ascontiguousarray(inputs[i], dtype=np.float32),
            "W_q": np.asarray(W_q, dtype=np.float32),
            "W_k": np.asarray(W_k, dtype=np.float32),
            "W_v": np.asarray(W_v, dtype=np.float32),
            "W_o": np.asarray(W_o, dtype=np.float32),
            "b_o": np.asarray(b_o, dtype=np.float32),
        }
        for i in range(B)
    ]


def run_sharded(in_maps, trace=False, **kw):
    nc = _get_nc()
    return run_bass_kernel_spmd(nc, in_maps, core_ids=list(range(B)), trace=trace, **kw)


def kernel(inputs, W_q, W_k, W_v, W_o, b_o):
    inputs = np.asarray(inputs)
    res = run_sharded(make_in_maps(inputs, W_q, W_k, W_v, W_o, b_o))
    out = np.stack([np.asarray(res.results[i]["out"]) for i in range(B)], axis=0)
    return out.astype(np.float32)


if __name__ == "__main__":
    rng = np.random.default_rng(0)
    ins = {
        "inputs": rng.standard_normal((B, S, D), dtype=np.float32),
        "W_q": rng.standard_normal((D, U), dtype=np.float32) / 16.0,
        "W_k": rng.standard_normal((D, U), dtype=np.float32) / 16.0,
        "W_v": rng.standard_normal((D, U), dtype=np.float32) / 16.0,
        "W_o": rng.standard_normal((U, D), dtype=np.float32) / np.sqrt(128.0),
        "b_o": np.zeros((D,), dtype=np.float32),
    }
    out = kernel(**ins)
    print("out", out.shape, out.dtype, float(np.abs(out).mean()))
